# revision 33
# baseline (speedup 1.0000x reference)
"""Trainium2 Bass kernel for nn_DeformNet (dense per-point MLP network).

Same contract and host packing as kernel.py, but the device program splits
the 1024 points into two fully independent halves and interleaves their
layer emissions, so one stream's matmuls fill the other stream's PSUM-drain
waits. This keeps the PE busy enough for the HAM clock to stay at 2.4 GHz.
"""

import os
import sys

for _p in ("/opt/trn_rl_repo", "/root/.axon_site/_ro/trn_rl_repo"):
    if _p not in sys.path and os.path.isdir(_p):
        sys.path.append(_p)

import numpy as np

import concourse.bass as bass
import concourse.tile as tile
from concourse import bacc, mybir
from concourse.bass_utils import run_bass_kernel_spmd

F32 = mybir.dt.float32
F32R = mybir.dt.float32r
RELU = mybir.ActivationFunctionType.Relu
COPY = mybir.ActivationFunctionType.Identity
ADD = mybir.AluOpType.add
MAX = mybir.AluOpType.max

NPTS = 1024
NV = 1024
NCAT = 6
NHALF = 512  # fp32 moving-operand max per matmul

WSM_W = 1000  # 704 weight cols + 37 bias cols + bias-row regions
BIAS_BASE = 704

# bias column map (columns BIAS_BASE+c of the wsm tile)
BIAS_COLS = {
    "p1": 0, "p2": 1, "p3": 2,   # paired ig/cl biases: rows 0:64 ig, 64:128 cl
    "ic": 3, "cg": 4, "iglob": 5,
    "t64a": 6,    # 4 cols
    "t64b": 10,
    "t128a": 11,  # 4 cols
    "t128b": 15,
    "as0": 16,    # 4 cols
    "as1": 20,    # 2 cols
    "as2": 22,    # 8 cols
    "de0": 30,    # 4 cols
    "de1": 34,    # 2 cols
    "de2": 36,
}
# row-0 bias rows (for K=1 bias matmuls): [b_t64b, b_t64b] then b_t128b
T64B_BROW = 744
T128B_BROW = 872

NS = 512  # stream width

_PROGRAM = None


def _build_program():
    nc = bacc.Bacc("TRN2", target_bir_lowering=False, debug=False, num_devices=8)

    dram_in = {}
    for name, shape in [
        ("xin", [39, NPTS]),
        ("wsm", [128, WSM_W]),
        ("wt64a", [128, 512]), ("wt64b", [128, 512]),
        ("wt128a", [128, 1024]), ("wt128b", [128, 512]),
        ("was0", [128, 512]), ("was1", [128, 1024]), ("was2", [128, 2048]),
        ("wde0", [128, 512]), ("wde1", [128, 1024]), ("wde2", [128, 6]),
    ]:
        dram_in[name] = nc.dram_tensor(name, shape, F32, kind="ExternalInput")

    out_a = nc.dram_tensor("assign_T", [NV, NPTS], F32, kind="ExternalOutput")
    out_d = nc.dram_tensor("deltas_T", [3, NPTS], F32, kind="ExternalOutput")

    with tile.TileContext(nc) as tc:
        with (
            tc.tile_pool(name="w", bufs=1) as wp,
            tc.tile_pool(name="a", bufs=1) as ap_,
            tc.tile_pool(name="hb", bufs=2) as hp,
            tc.tile_pool(name="ps", bufs=8, space="PSUM") as pp,
            tc.tile_pool(name="o", bufs=4) as op_,
        ):
            wsm_t = wp.tile([128, WSM_W], F32R, tag="wsm")
            nc.sync.dma_start(wsm_t[:], dram_in["wsm"][:].bitcast(F32R))
            xpp = wp.tile([6, NPTS], F32R, tag="xpp")
            xemb = wp.tile([32, NPTS], F32R, tag="xemb")
            xones = wp.tile([1, NPTS], F32R, tag="xones")
            nc.sync.dma_start(xpp[:], dram_in["xin"][0:6, :].bitcast(F32R))
            nc.sync.dma_start(xemb[:], dram_in["xin"][6:38, :].bitcast(F32R))
            nc.sync.dma_start(xones[:], dram_in["xin"][38:39, :].bitcast(F32R))

            wz = wp.tile([128, NS], F32, tag="wz")
            nc.vector.memset(wz[:], 0.0)
            for i in range(5):
                psw = pp.tile([128, NS], F32, tag="ps")
                nc.tensor.matmul(psw[:], wz[:, 0:128], wz[:],
                                 start=True, stop=True)

            DMA_ENG = {
                "wt64a": nc.scalar, "wt64b": nc.scalar,
                "wt128a": nc.scalar, "wt128b": nc.scalar,
                "was0": nc.gpsimd, "wde0": nc.gpsimd,
                "was1": nc.gpsimd, "wde1": nc.gpsimd,
                "was2": nc.sync, "wde2": nc.sync,
            }
            W = {"wsm": wsm_t}
            for name in ("wt64a", "wt64b", "wt128a", "wt128b",
                         "was0", "wde0", "was1", "wde1", "was2", "wde2"):
                cols = dram_in[name].shape[1]
                t = wp.tile([128, cols], F32R, tag=name)
                DMA_ENG[name].dma_start(t[:], dram_in[name][:].bitcast(F32R))
                W[name] = t

            wsmb = wsm_t[:].bitcast(F32)

            def bias(key, rows, off=0, prow=0):
                c = BIAS_BASE + BIAS_COLS[key] + off
                return wsmb[prow:prow + rows, c:c + 1]

            def mm(ps, lhsT, rhs, start=True, stop=True):
                nc.tensor.matmul(ps, lhsT, rhs, start=start, stop=stop)

            def drain(ps, dst, b, relu, eng):
                # eng 0 -> ACT, 1 -> DVE
                if eng == 0:
                    nc.scalar.activation(dst, ps, RELU if relu else COPY,
                                         bias=b)
                elif relu:
                    nc.vector.tensor_scalar(dst, ps, b, 0.0, ADD, MAX)
                else:
                    nc.vector.tensor_scalar(dst, ps, b, None, ADD)

            def stream(s):
                """Generator emitting one scheduling quantum per yield."""
                sl = slice(s * NS, (s + 1) * NS)
                e = s  # drain-engine phase: stream 0 starts ACT, 1 starts DVE

                def eng():
                    nonlocal e
                    e ^= 1
                    return e

                ab1 = ap_.tile([128, NS], F32R, tag=f"ab1{s}")
                ab2 = ap_.tile([128, NS], F32R, tag=f"ab2{s}")
                bcl = ap_.tile([128, NS], F32R, tag=f"bcl{s}")
                tpe = ap_.tile([128, NS], F32R, tag=f"tpe{s}")
                ig = ap_.tile([128, NS], F32R, tag=f"ig{s}")
                cg = ap_.tile([128, NS], F32R, tag=f"cg{s}")
                qsb = ap_.tile([128, NS], F32, tag=f"qsb{s}")

                # pair1
                ps = pp.tile([128, NS], F32, tag="ps")
                mm(ps[:], wsm_t[0:6, 0:128], xpp[0:6, sl])
                drain(ps[:], ab1[:], bias("p1", 128), True, eng())
                yield
                # ic (ACT only: partition-shifted write)
                ps = pp.tile([64, NS], F32, tag="ps")
                mm(ps[:], wsm_t[0:32, 384:448], xemb[:, sl])
                drain(ps[:], tpe[64:128, :], bias("ic", 64), True, 0)
                yield
                # pair2
                ps = pp.tile([128, NS], F32, tag="ps")
                mm(ps[:], wsm_t[0:128, 128:256], ab1[:])
                drain(ps[:], ab2[:], bias("p2", 128), True, eng())
                yield
                # pair3: rows 0:64 -> tpe, rows 64:128 -> bcl
                ps = pp.tile([128, NS], F32, tag="ps")
                mm(ps[:], wsm_t[0:128, 256:384], ab2[:])
                drain(ps[0:64, :], tpe[0:64, :], bias("p3", 64), True, 0)
                nc.vector.tensor_scalar(bcl[64:128, :], ps[64:128, :],
                                        bias("p3", 64, prow=64), 0.0, ADD, MAX)
                yield
                # cg
                ps = pp.tile([128, NS], F32, tag="ps")
                mm(ps[:], wsm_t[64:128, 448:576], bcl[64:128, :])
                drain(ps[:], cg[:], bias("cg", 128), True, eng())
                yield
                # t64a
                h64 = hp.tile([128, 4, NS], F32R, tag=f"hbig{s}")
                for m in range(4):
                    ps = pp.tile([128, NS], F32, tag="ps")
                    mm(ps[:], W["wt64a"][:, m * 128:(m + 1) * 128], tpe[:])
                    drain(ps[:], h64[:, m, :], bias("t64a", 128, m), True, eng())
                    yield
                # t64b: bias row matmul + duplicated-M weights, then the
                # residual add straight from PSUM
                ps = pp.tile([128, NS], F32, tag="ps")
                mm(ps[:], wsm_t[0:1, T64B_BROW:T64B_BROW + 128], xones[:, sl],
                   start=True, stop=False)
                for k in range(4):
                    mm(ps[:], W["wt64b"][:, k * 128:(k + 1) * 128],
                       h64[:, k, :], start=False, stop=(k == 3))
                nc.vector.tensor_add(tpe[:].bitcast(F32R),
                                     tpe[:].bitcast(F32), ps[:])
                yield
                # iglob
                ps = pp.tile([128, NS], F32, tag="ps")
                mm(ps[:], wsm_t[:, 576:704], tpe[:])
                drain(ps[:], ig[:], bias("iglob", 128), True, eng())
                yield
                # t128a
                h128 = hp.tile([128, 4, NS], F32R, tag=f"hbig{s}")
                for m in range(4):
                    ps = pp.tile([128, NS], F32, tag="ps")
                    for k, src in ((0, ig), (1, cg)):
                        mm(ps[:],
                           W["wt128a"][:, k * 512 + m * 128:k * 512 + (m + 1) * 128],
                           src[:], start=(k == 0), stop=(k == 1))
                    drain(ps[:], h128[:, m, :], bias("t128a", 128, m), True,
                          eng())
                    yield
                # t128b + residual adds
                ps = pp.tile([128, NS], F32, tag="ps")
                mm(ps[:], wsm_t[0:1, T128B_BROW:T128B_BROW + 128], xones[:, sl],
                   start=True, stop=False)
                for k in range(4):
                    mm(ps[:], W["wt128b"][:, k * 128:(k + 1) * 128],
                       h128[:, k, :], start=False, stop=(k == 3))
                nc.vector.tensor_add(ig[:].bitcast(F32R), ig[:].bitcast(F32),
                                     ps[:])
                nc.scalar.activation(qsb[:], ps[:], COPY, bias=0.0)
                nc.gpsimd.tensor_add(cg[:].bitcast(F32R), cg[:].bitcast(F32),
                                     qsb[:])
                yield
                # heads
                ah1 = hp.tile([128, 4, NS], F32R, tag=f"hbig{s}")
                dh1 = hp.tile([128, 4, NS], F32R, tag=f"hbig{s}")
                ah2 = hp.tile([128, 2, NS], F32R, tag=f"h2{s}")
                dh2 = hp.tile([128, 2, NS], F32R, tag=f"h2{s}")

                for m in range(4):
                    ps = pp.tile([128, NS], F32, tag="ps")
                    mm(ps[:], W["was0"][:, m * 128:(m + 1) * 128], ig[:])
                    drain(ps[:], ah1[:, m, :], bias("as0", 128, m), True, eng())
                    yield
                for m in range(2):
                    ps = pp.tile([128, NS], F32, tag="ps")
                    for k in range(4):
                        mm(ps[:],
                           W["was1"][:, k * 256 + m * 128:k * 256 + (m + 1) * 128],
                           ah1[:, k, :], start=(k == 0), stop=(k == 3))
                    drain(ps[:], ah2[:, m, :], bias("as1", 128, m), True, eng())
                    yield
                for m in range(8):
                    ps = pp.tile([128, NS], F32, tag="ps")
                    for k in range(2):
                        mm(ps[:],
                           W["was2"][:, k * 1024 + m * 128:k * 1024 + (m + 1) * 128],
                           ah2[:, k, :], start=(k == 0), stop=(k == 1))
                    ot = op_.tile([128, NS], F32, tag="oa")
                    drain(ps[:], ot[:], bias("as2", 128, m), False, eng())
                    # split output issues across the two HWDGE-capable
                    # sequencers (gpsimd/SWDGE wedges the device here)
                    out_eng = nc.sync if s == 0 else nc.scalar
                    out_eng.dma_start(out_a[m * 128:(m + 1) * 128, sl], ot[:])
                    yield
                for m in range(4):
                    ps = pp.tile([128, NS], F32, tag="ps")
                    mm(ps[:], W["wde0"][:, m * 128:(m + 1) * 128], cg[:])
                    drain(ps[:], dh1[:, m, :], bias("de0", 128, m), True, eng())
                    yield
                for m in range(2):
                    ps = pp.tile([128, NS], F32, tag="ps")
                    for k in range(4):
                        mm(ps[:],
                           W["wde1"][:, k * 256 + m * 128:k * 256 + (m + 1) * 128],
                           dh1[:, k, :], start=(k == 0), stop=(k == 3))
                    drain(ps[:], dh2[:, m, :], bias("de1", 128, m), True, eng())
                    yield
                ps = pp.tile([3, NS], F32, tag="ps")
                for k in range(2):
                    mm(ps[:], W["wde2"][:, k * 3:(k + 1) * 3], dh2[:, k, :],
                       start=(k == 0), stop=(k == 1))
                od = op_.tile([3, NS], F32, tag="od")
                nc.vector.tensor_scalar(od[:], ps[:], bias("de2", 3), None, ADD)
                nc.sync.dma_start(out_d[:, sl], od[:])
                yield

            g0, g1 = stream(0), stream(1)
            alive = [g0, g1]
            rnd = 0
            while alive:
                for g in list(alive):
                    try:
                        next(g)
                    except StopIteration:
                        alive.remove(g)
                # trickle dummy matmuls through the latency-bound chain
                # region so the PE activity window stays primed (HAM warm)
                if rnd < 8:
                    psw = pp.tile([128, NS], F32, tag="ps")
                    nc.tensor.matmul(psw[:], wz[:, 0:128], wz[:],
                                     start=True, stop=True)
                rnd += 1

    nc.compile()
    return nc


def _get_program():
    global _PROGRAM
    if _PROGRAM is None:
        _PROGRAM = _build_program()
    return _PROGRAM


def _pack_blocks(wt, block_cols):
    """[K, M] with K = nk*128 -> [128, nk*M] (K-tile blocks side by side)."""
    K, M = wt.shape
    nk = K // 128
    assert nk * 128 == K and M == block_cols
    return np.concatenate([wt[i * 128:(i + 1) * 128, :] for i in range(nk)], axis=1)


def _np(x, dtype=None):
    try:
        a = np.asarray(x)
    except Exception:
        import jax
        a = np.asarray(jax.device_get(x))
    return a.astype(dtype) if dtype is not None and a.dtype != dtype else a


def _host_pack(points, emb_map, choose, cat_id, prior, params):
    """Build the 8 per-core input maps."""
    p = {k: [(_np(w, np.float32), _np(b, np.float32)) for w, b in v]
         for k, v in params.items()}

    def wT(key, i):
        return np.ascontiguousarray(p[key][i][0].T)

    wsm = np.zeros((128, WSM_W), np.float32)
    # block-diagonal pairs: rows 0:K_ig cols 0:64 = ig_i, rows K.. cols 64:128 = cl_i
    wsm[0:3, 0:64] = wT("ig", 0)
    wsm[3:6, 64:128] = wT("cl", 0)
    wsm[0:64, 128:192] = wT("ig", 1)
    wsm[64:128, 192:256] = wT("cl", 1)
    wsm[0:64, 256:320] = wT("ig", 2)
    wsm[64:128, 320:384] = wT("cl", 2)
    wsm[0:32, 384:448] = wT("ic", 0)
    wsm[64:128, 448:576] = wT("cg", 0)   # rows 64:128: rhs lives there too
    wsm[0:128, 576:704] = wT("iglob", 0)

    def put_bias(vec, col, prow=0):
        vec = np.asarray(vec, np.float32).reshape(-1)
        wsm[prow:prow + vec.size, BIAS_BASE + col] = vec

    put_bias(np.concatenate([p["ig"][0][1], p["cl"][0][1]]), BIAS_COLS["p1"])
    put_bias(np.concatenate([p["ig"][1][1], p["cl"][1][1]]), BIAS_COLS["p2"])
    put_bias(np.concatenate([p["ig"][2][1], p["cl"][2][1]]), BIAS_COLS["p3"])
    put_bias(p["ic"][0][1], BIAS_COLS["ic"])
    put_bias(p["cg"][0][1], BIAS_COLS["cg"])
    put_bias(p["iglob"][0][1], BIAS_COLS["iglob"])

    def put_bias_tiles(key, i, col):
        b = p[key][i][1]
        n = b.size // 128
        for j in range(n):
            put_bias(b[j * 128:(j + 1) * 128], col + j)

    put_bias_tiles("t64", 0, BIAS_COLS["t64a"])
    put_bias_tiles("t128", 0, BIAS_COLS["t128a"])
    # bias ROWS (row 0) for the K=1 bias matmuls
    b64 = p["t64"][1][1].reshape(-1)
    wsm[0, T64B_BROW:T64B_BROW + 128] = np.concatenate([b64, b64])
    wsm[0, T128B_BROW:T128B_BROW + 128] = p["t128"][1][1].reshape(-1)
    put_bias_tiles("assign", 0, BIAS_COLS["as0"])
    put_bias_tiles("assign", 1, BIAS_COLS["as1"])
    put_bias_tiles("deform", 0, BIAS_COLS["de0"])
    put_bias_tiles("deform", 1, BIAS_COLS["de1"])

    wt64b_T = wT("t64", 1)  # (512, 64)
    wt64b_dup = np.concatenate([wt64b_T, wt64b_T], axis=1)  # (512, 128)
    base = {
        "wt64a": wT("t64", 0),
        "wt64b": _pack_blocks(wt64b_dup, 128),
        "wt128a": _pack_blocks(wT("t128", 0), 512),
        "wt128b": _pack_blocks(wT("t128", 1), 128),
        "was0": wT("assign", 0),
        "was1": _pack_blocks(wT("assign", 1), 256),
        "wde0": wT("deform", 0),
        "wde1": _pack_blocks(wT("deform", 1), 256),
    }

    points = _np(points, np.float32)
    prior = _np(prior, np.float32)
    emb_map = _np(emb_map, np.float32)
    choose = _np(choose, np.int64)
    cat_id = _np(cat_id, np.int64)

    was2_w = p["assign"][2][0]   # (6144, 256)
    was2_b = p["assign"][2][1]   # (6144,)
    wde2_w = p["deform"][2][0]   # (18, 256)
    wde2_b = p["deform"][2][1]   # (18,)

    in_maps = []
    for i in range(8):
        cat = int(cat_id[i])
        wsmi = wsm.copy()
        b8 = was2_b[cat * NV:(cat + 1) * NV].reshape(8, 128)
        for j in range(8):
            wsmi[0:128, BIAS_BASE + BIAS_COLS["as2"] + j] = b8[j]
        wsmi[0:3, BIAS_BASE + BIAS_COLS["de2"]] = wde2_b[cat * 3:cat * 3 + 3]

        xin = np.empty((39, NPTS), np.float32)
        xin[0:3] = points[i].T
        xin[3:6] = prior[i].T
        xin[6:38] = emb_map[i][:, choose[i]]
        xin[38] = 1.0

        m = dict(base)
        m["wsm"] = wsmi
        m["xin"] = xin
        m["was2"] = _pack_blocks(
            np.ascontiguousarray(was2_w[cat * NV:(cat + 1) * NV, :].T), NV)
        m["wde2"] = _pack_blocks(
            np.ascontiguousarray(wde2_w[cat * 3:cat * 3 + 3, :].T), 3)
        in_maps.append(m)
    return in_maps


# Optional override used by test.py to run with NTFF profiling; the graded
# path never sets this.
RUNNER = None


def _assemble(results):
    assign = np.stack([results[i]["assign_T"].T for i in range(8)])
    deltas = np.stack([results[i]["deltas_T"].T for i in range(8)])
    return np.ascontiguousarray(assign), np.ascontiguousarray(deltas)


def kernel(points, emb_map, choose, cat_id, prior, params):
    nc = _get_program()
    in_maps = _host_pack(points, emb_map, choose, cat_id, prior, params)
    if RUNNER is not None:
        results = RUNNER(nc, in_maps)
    else:
        results = run_bass_kernel_spmd(nc, in_maps, list(range(8))).results
    return _assemble(results)


# revision 34
# speedup vs baseline: 1.2128x; 1.2128x over previous
"""Trainium2 Bass kernel for nn_DeformNet (dense per-point MLP network).

Same contract and host packing as kernel.py, but the device program splits
the 1024 points into two fully independent halves and interleaves their
layer emissions, so one stream's matmuls fill the other stream's PSUM-drain
waits. This keeps the PE busy enough for the HAM clock to stay at 2.4 GHz.
"""

import os
import sys

for _p in ("/opt/trn_rl_repo", "/root/.axon_site/_ro/trn_rl_repo"):
    if _p not in sys.path and os.path.isdir(_p):
        sys.path.append(_p)

import numpy as np

import concourse.bass as bass
import concourse.tile as tile
from concourse import bacc, mybir
from concourse.bass_utils import run_bass_kernel_spmd

F32 = mybir.dt.float32
F32R = mybir.dt.float32r
RELU = mybir.ActivationFunctionType.Relu
COPY = mybir.ActivationFunctionType.Identity
ADD = mybir.AluOpType.add
MAX = mybir.AluOpType.max

NPTS = 1024
NV = 1024
NCAT = 6
NHALF = 512  # fp32 moving-operand max per matmul

WSM_W = 1000  # 704 weight cols + 37 bias cols + bias-row regions
BIAS_BASE = 704

# bias column map (columns BIAS_BASE+c of the wsm tile)
BIAS_COLS = {
    "p1": 0, "p2": 1, "p3": 2,   # paired ig/cl biases: rows 0:64 ig, 64:128 cl
    "ic": 3, "cg": 4, "iglob": 5,
    "t64a": 6,    # 4 cols
    "t64b": 10,
    "t128a": 11,  # 4 cols
    "t128b": 15,
    "as0": 16,    # 4 cols
    "as1": 20,    # 2 cols
    "as2": 22,    # 8 cols
    "de0": 30,    # 4 cols
    "de1": 34,    # 2 cols
    "de2": 36,
}
# row-0 bias rows (for K=1 bias matmuls): [b_t64b, b_t64b] then b_t128b
T64B_BROW = 744
T128B_BROW = 872

NS = 512  # stream width

_PROGRAM = None


def _build_program():
    nc = bacc.Bacc("TRN2", target_bir_lowering=False, debug=False, num_devices=8)

    dram_in = {}
    for name, shape in [
        ("xin", [39, NPTS]),
        ("wsm", [128, WSM_W]),
        ("wt64a", [128, 512]), ("wt64b", [128, 512]),
        ("wt128a", [128, 1024]), ("wt128b", [128, 512]),
        ("was0", [128, 512]), ("was1", [128, 1024]), ("was2", [128, 2048]),
        ("wde0", [128, 512]), ("wde1", [128, 1024]), ("wde2", [128, 6]),
    ]:
        dram_in[name] = nc.dram_tensor(name, shape, F32, kind="ExternalInput")

    out_a = nc.dram_tensor("assign_T", [NV, NPTS], F32, kind="ExternalOutput")
    out_d = nc.dram_tensor("deltas_T", [3, NPTS], F32, kind="ExternalOutput")

    with tile.TileContext(nc) as tc:
        with (
            tc.tile_pool(name="w", bufs=1) as wp,
            tc.tile_pool(name="a", bufs=1) as ap_,
            tc.tile_pool(name="hb", bufs=2) as hp,
            tc.tile_pool(name="ps", bufs=8, space="PSUM") as pp,
            tc.tile_pool(name="o", bufs=4) as op_,
        ):
            wsm_t = wp.tile([128, WSM_W], F32R, tag="wsm")
            nc.sync.dma_start(wsm_t[:], dram_in["wsm"][:].bitcast(F32R))
            xpp = wp.tile([6, NPTS], F32R, tag="xpp")
            xemb = wp.tile([32, NPTS], F32R, tag="xemb")
            xones = wp.tile([1, NPTS], F32R, tag="xones")
            nc.sync.dma_start(xpp[:], dram_in["xin"][0:6, :].bitcast(F32R))
            nc.sync.dma_start(xemb[:], dram_in["xin"][6:38, :].bitcast(F32R))
            nc.sync.dma_start(xones[:], dram_in["xin"][38:39, :].bitcast(F32R))

            wz = wp.tile([128, NS], F32, tag="wz")
            nc.vector.memset(wz[:], 0.0)
            for i in range(5):
                psw = pp.tile([128, NS], F32, tag="ps")
                nc.tensor.matmul(psw[:], wz[:, 0:128], wz[:],
                                 start=True, stop=True)

            DMA_ENG = {
                "wt64a": nc.scalar, "wt64b": nc.scalar,
                "wt128a": nc.scalar, "wt128b": nc.scalar,
                "was0": nc.gpsimd, "wde0": nc.gpsimd,
                "was1": nc.gpsimd, "wde1": nc.gpsimd,
                "was2": nc.sync, "wde2": nc.sync,
            }
            W = {"wsm": wsm_t}
            for name in ("wt64a", "wt64b", "wt128a", "wt128b",
                         "was0", "wde0", "was1", "wde1", "was2", "wde2"):
                cols = dram_in[name].shape[1]
                t = wp.tile([128, cols], F32R, tag=name)
                DMA_ENG[name].dma_start(t[:], dram_in[name][:].bitcast(F32R))
                W[name] = t

            wsmb = wsm_t[:].bitcast(F32)

            def bias(key, rows, off=0, prow=0):
                c = BIAS_BASE + BIAS_COLS[key] + off
                return wsmb[prow:prow + rows, c:c + 1]

            def mm(ps, lhsT, rhs, start=True, stop=True):
                nc.tensor.matmul(ps, lhsT, rhs, start=start, stop=stop)

            def drain(ps, dst, b, relu, eng):
                # eng 0 -> ACT, 1 -> DVE
                if eng == 0:
                    nc.scalar.activation(dst, ps, RELU if relu else COPY,
                                         bias=b)
                elif relu:
                    nc.vector.tensor_scalar(dst, ps, b, 0.0, ADD, MAX)
                else:
                    nc.vector.tensor_scalar(dst, ps, b, None, ADD)

            def stream(s):
                """Generator emitting one scheduling quantum per yield."""
                sl = slice(s * NS, (s + 1) * NS)
                e = s  # drain-engine phase: stream 0 starts ACT, 1 starts DVE

                def eng():
                    nonlocal e
                    e ^= 1
                    return e

                ab1 = ap_.tile([128, NS], F32R, tag=f"ab1{s}")
                ab2 = ap_.tile([128, NS], F32R, tag=f"ab2{s}")
                bcl = ap_.tile([128, NS], F32R, tag=f"bcl{s}")
                tpe = ap_.tile([128, NS], F32R, tag=f"tpe{s}")
                ig = ap_.tile([128, NS], F32R, tag=f"ig{s}")
                cg = ap_.tile([128, NS], F32R, tag=f"cg{s}")
                qsb = ap_.tile([128, NS], F32, tag=f"qsb{s}")

                # pair1
                ps = pp.tile([128, NS], F32, tag="ps")
                mm(ps[:], wsm_t[0:6, 0:128], xpp[0:6, sl])
                drain(ps[:], ab1[:], bias("p1", 128), True, eng())
                yield
                # ic (ACT only: partition-shifted write)
                ps = pp.tile([64, NS], F32, tag="ps")
                mm(ps[:], wsm_t[0:32, 384:448], xemb[:, sl])
                drain(ps[:], tpe[64:128, :], bias("ic", 64), True, 0)
                yield
                # pair2
                ps = pp.tile([128, NS], F32, tag="ps")
                mm(ps[:], wsm_t[0:128, 128:256], ab1[:])
                drain(ps[:], ab2[:], bias("p2", 128), True, eng())
                yield
                # pair3: rows 0:64 -> tpe, rows 64:128 -> bcl
                ps = pp.tile([128, NS], F32, tag="ps")
                mm(ps[:], wsm_t[0:128, 256:384], ab2[:])
                drain(ps[0:64, :], tpe[0:64, :], bias("p3", 64), True, 0)
                nc.vector.tensor_scalar(bcl[64:128, :], ps[64:128, :],
                                        bias("p3", 64, prow=64), 0.0, ADD, MAX)
                yield
                # cg
                ps = pp.tile([128, NS], F32, tag="ps")
                mm(ps[:], wsm_t[64:128, 448:576], bcl[64:128, :])
                drain(ps[:], cg[:], bias("cg", 128), True, eng())
                yield
                # t64a
                h64 = hp.tile([128, 4, NS], F32R, tag=f"hbig{s}")
                for m in range(4):
                    ps = pp.tile([128, NS], F32, tag="ps")
                    mm(ps[:], W["wt64a"][:, m * 128:(m + 1) * 128], tpe[:])
                    drain(ps[:], h64[:, m, :], bias("t64a", 128, m), True, eng())
                    yield
                # t64b: bias row matmul + duplicated-M weights, then the
                # residual add straight from PSUM
                ps = pp.tile([128, NS], F32, tag="ps")
                mm(ps[:], wsm_t[0:1, T64B_BROW:T64B_BROW + 128], xones[:, sl],
                   start=True, stop=False)
                for k in range(4):
                    mm(ps[:], W["wt64b"][:, k * 128:(k + 1) * 128],
                       h64[:, k, :], start=False, stop=(k == 3))
                nc.vector.tensor_add(tpe[:].bitcast(F32R),
                                     tpe[:].bitcast(F32), ps[:])
                yield
                # iglob
                ps = pp.tile([128, NS], F32, tag="ps")
                mm(ps[:], wsm_t[:, 576:704], tpe[:])
                drain(ps[:], ig[:], bias("iglob", 128), True, eng())
                yield
                # t128a
                h128 = hp.tile([128, 4, NS], F32R, tag=f"hbig{s}")
                for m in range(4):
                    ps = pp.tile([128, NS], F32, tag="ps")
                    for k, src in ((0, ig), (1, cg)):
                        mm(ps[:],
                           W["wt128a"][:, k * 512 + m * 128:k * 512 + (m + 1) * 128],
                           src[:], start=(k == 0), stop=(k == 1))
                    drain(ps[:], h128[:, m, :], bias("t128a", 128, m), True,
                          eng())
                    yield
                # t128b + residual adds
                ps = pp.tile([128, NS], F32, tag="ps")
                mm(ps[:], wsm_t[0:1, T128B_BROW:T128B_BROW + 128], xones[:, sl],
                   start=True, stop=False)
                for k in range(4):
                    mm(ps[:], W["wt128b"][:, k * 128:(k + 1) * 128],
                       h128[:, k, :], start=False, stop=(k == 3))
                nc.vector.tensor_add(ig[:].bitcast(F32R), ig[:].bitcast(F32),
                                     ps[:])
                nc.scalar.activation(qsb[:], ps[:], COPY, bias=0.0)
                nc.gpsimd.tensor_add(cg[:].bitcast(F32R), cg[:].bitcast(F32),
                                     qsb[:])
                yield
                # heads
                ah1 = hp.tile([128, 4, NS], F32R, tag=f"hbig{s}")
                dh1 = hp.tile([128, 4, NS], F32R, tag=f"hbig{s}")
                ah2 = hp.tile([128, 2, NS], F32R, tag=f"h2{s}")
                dh2 = hp.tile([128, 2, NS], F32R, tag=f"h2{s}")

                for m in range(4):
                    ps = pp.tile([128, NS], F32, tag="ps")
                    mm(ps[:], W["was0"][:, m * 128:(m + 1) * 128], ig[:])
                    drain(ps[:], ah1[:, m, :], bias("as0", 128, m), True, eng())
                    yield
                for m in range(2):
                    ps = pp.tile([128, NS], F32, tag="ps")
                    for k in range(4):
                        mm(ps[:],
                           W["was1"][:, k * 256 + m * 128:k * 256 + (m + 1) * 128],
                           ah1[:, k, :], start=(k == 0), stop=(k == 3))
                    drain(ps[:], ah2[:, m, :], bias("as1", 128, m), True, eng())
                    yield
                for m in range(8):
                    ps = pp.tile([128, NS], F32, tag="ps")
                    for k in range(2):
                        mm(ps[:],
                           W["was2"][:, k * 1024 + m * 128:k * 1024 + (m + 1) * 128],
                           ah2[:, k, :], start=(k == 0), stop=(k == 1))
                    ot = op_.tile([128, NS], F32, tag="oa")
                    drain(ps[:], ot[:], bias("as2", 128, m), False, eng())
                    nc.sync.dma_start(out_a[m * 128:(m + 1) * 128, sl], ot[:])
                    yield
                for m in range(4):
                    ps = pp.tile([128, NS], F32, tag="ps")
                    mm(ps[:], W["wde0"][:, m * 128:(m + 1) * 128], cg[:])
                    drain(ps[:], dh1[:, m, :], bias("de0", 128, m), True, eng())
                    yield
                for m in range(2):
                    ps = pp.tile([128, NS], F32, tag="ps")
                    for k in range(4):
                        mm(ps[:],
                           W["wde1"][:, k * 256 + m * 128:k * 256 + (m + 1) * 128],
                           dh1[:, k, :], start=(k == 0), stop=(k == 3))
                    drain(ps[:], dh2[:, m, :], bias("de1", 128, m), True, eng())
                    yield
                ps = pp.tile([3, NS], F32, tag="ps")
                for k in range(2):
                    mm(ps[:], W["wde2"][:, k * 3:(k + 1) * 3], dh2[:, k, :],
                       start=(k == 0), stop=(k == 1))
                od = op_.tile([3, NS], F32, tag="od")
                nc.vector.tensor_scalar(od[:], ps[:], bias("de2", 3), None, ADD)
                nc.sync.dma_start(out_d[:, sl], od[:])
                yield

            g0, g1 = stream(0), stream(1)
            alive = [g0, g1]
            rnd = 0
            while alive:
                for g in list(alive):
                    try:
                        next(g)
                    except StopIteration:
                        alive.remove(g)
                # trickle dummy matmuls through the latency-bound chain
                # region so the PE activity window stays primed (HAM warm)
                if rnd < 8:
                    psw = pp.tile([128, NS], F32, tag="ps")
                    nc.tensor.matmul(psw[:], wz[:, 0:128], wz[:],
                                     start=True, stop=True)
                rnd += 1

    nc.compile()
    return nc


def _get_program():
    global _PROGRAM
    if _PROGRAM is None:
        _PROGRAM = _build_program()
    return _PROGRAM


def _pack_blocks(wt, block_cols):
    """[K, M] with K = nk*128 -> [128, nk*M] (K-tile blocks side by side)."""
    K, M = wt.shape
    nk = K // 128
    assert nk * 128 == K and M == block_cols
    return np.concatenate([wt[i * 128:(i + 1) * 128, :] for i in range(nk)], axis=1)


def _np(x, dtype=None):
    try:
        a = np.asarray(x)
    except Exception:
        import jax
        a = np.asarray(jax.device_get(x))
    return a.astype(dtype) if dtype is not None and a.dtype != dtype else a


def _host_pack(points, emb_map, choose, cat_id, prior, params):
    """Build the 8 per-core input maps."""
    p = {k: [(_np(w, np.float32), _np(b, np.float32)) for w, b in v]
         for k, v in params.items()}

    def wT(key, i):
        return np.ascontiguousarray(p[key][i][0].T)

    wsm = np.zeros((128, WSM_W), np.float32)
    # block-diagonal pairs: rows 0:K_ig cols 0:64 = ig_i, rows K.. cols 64:128 = cl_i
    wsm[0:3, 0:64] = wT("ig", 0)
    wsm[3:6, 64:128] = wT("cl", 0)
    wsm[0:64, 128:192] = wT("ig", 1)
    wsm[64:128, 192:256] = wT("cl", 1)
    wsm[0:64, 256:320] = wT("ig", 2)
    wsm[64:128, 320:384] = wT("cl", 2)
    wsm[0:32, 384:448] = wT("ic", 0)
    wsm[64:128, 448:576] = wT("cg", 0)   # rows 64:128: rhs lives there too
    wsm[0:128, 576:704] = wT("iglob", 0)

    def put_bias(vec, col, prow=0):
        vec = np.asarray(vec, np.float32).reshape(-1)
        wsm[prow:prow + vec.size, BIAS_BASE + col] = vec

    put_bias(np.concatenate([p["ig"][0][1], p["cl"][0][1]]), BIAS_COLS["p1"])
    put_bias(np.concatenate([p["ig"][1][1], p["cl"][1][1]]), BIAS_COLS["p2"])
    put_bias(np.concatenate([p["ig"][2][1], p["cl"][2][1]]), BIAS_COLS["p3"])
    put_bias(p["ic"][0][1], BIAS_COLS["ic"])
    put_bias(p["cg"][0][1], BIAS_COLS["cg"])
    put_bias(p["iglob"][0][1], BIAS_COLS["iglob"])

    def put_bias_tiles(key, i, col):
        b = p[key][i][1]
        n = b.size // 128
        for j in range(n):
            put_bias(b[j * 128:(j + 1) * 128], col + j)

    put_bias_tiles("t64", 0, BIAS_COLS["t64a"])
    put_bias_tiles("t128", 0, BIAS_COLS["t128a"])
    # bias ROWS (row 0) for the K=1 bias matmuls
    b64 = p["t64"][1][1].reshape(-1)
    wsm[0, T64B_BROW:T64B_BROW + 128] = np.concatenate([b64, b64])
    wsm[0, T128B_BROW:T128B_BROW + 128] = p["t128"][1][1].reshape(-1)
    put_bias_tiles("assign", 0, BIAS_COLS["as0"])
    put_bias_tiles("assign", 1, BIAS_COLS["as1"])
    put_bias_tiles("deform", 0, BIAS_COLS["de0"])
    put_bias_tiles("deform", 1, BIAS_COLS["de1"])

    wt64b_T = wT("t64", 1)  # (512, 64)
    wt64b_dup = np.concatenate([wt64b_T, wt64b_T], axis=1)  # (512, 128)
    base = {
        "wt64a": wT("t64", 0),
        "wt64b": _pack_blocks(wt64b_dup, 128),
        "wt128a": _pack_blocks(wT("t128", 0), 512),
        "wt128b": _pack_blocks(wT("t128", 1), 128),
        "was0": wT("assign", 0),
        "was1": _pack_blocks(wT("assign", 1), 256),
        "wde0": wT("deform", 0),
        "wde1": _pack_blocks(wT("deform", 1), 256),
    }

    points = _np(points, np.float32)
    prior = _np(prior, np.float32)
    emb_map = _np(emb_map, np.float32)
    choose = _np(choose, np.int64)
    cat_id = _np(cat_id, np.int64)

    was2_w = p["assign"][2][0]   # (6144, 256)
    was2_b = p["assign"][2][1]   # (6144,)
    wde2_w = p["deform"][2][0]   # (18, 256)
    wde2_b = p["deform"][2][1]   # (18,)

    in_maps = []
    for i in range(8):
        cat = int(cat_id[i])
        wsmi = wsm.copy()
        b8 = was2_b[cat * NV:(cat + 1) * NV].reshape(8, 128)
        for j in range(8):
            wsmi[0:128, BIAS_BASE + BIAS_COLS["as2"] + j] = b8[j]
        wsmi[0:3, BIAS_BASE + BIAS_COLS["de2"]] = wde2_b[cat * 3:cat * 3 + 3]

        xin = np.empty((39, NPTS), np.float32)
        xin[0:3] = points[i].T
        xin[3:6] = prior[i].T
        xin[6:38] = emb_map[i][:, choose[i]]
        xin[38] = 1.0

        m = dict(base)
        m["wsm"] = wsmi
        m["xin"] = xin
        m["was2"] = _pack_blocks(
            np.ascontiguousarray(was2_w[cat * NV:(cat + 1) * NV, :].T), NV)
        m["wde2"] = _pack_blocks(
            np.ascontiguousarray(wde2_w[cat * 3:cat * 3 + 3, :].T), 3)
        in_maps.append(m)
    return in_maps


# Optional override used by test.py to run with NTFF profiling; the graded
# path never sets this.
RUNNER = None


def _assemble(results):
    assign = np.stack([results[i]["assign_T"].T for i in range(8)])
    deltas = np.stack([results[i]["deltas_T"].T for i in range(8)])
    return np.ascontiguousarray(assign), np.ascontiguousarray(deltas)


def kernel(points, emb_map, choose, cat_id, prior, params):
    nc = _get_program()
    in_maps = _host_pack(points, emb_map, choose, cat_id, prior, params)
    if RUNNER is not None:
        results = RUNNER(nc, in_maps)
    else:
        results = run_bass_kernel_spmd(nc, in_maps, list(range(8))).results
    return _assemble(results)
